# Initial kernel scaffold
#
"""NatPN radial-flow posterior kernel for Trainium2, 8 NeuronCores (SPMD).

Strategy (class-sharded "expert parallel" + sample-sharded epilogue):
  * The radial flow  z' = z + beta*h*(z - z0),  h = 1/(alpha + r), r = |z - z0|
    is reformulated in coefficient space:  z_t = A_t * (x - sum_j u_j z0_j).
    Per (class, transform) step the device only tracks per-sample scalars
    (q = |z|^2, A, running products P = prod(1+bh), Q = prod(1+alpha*beta*h^2))
    and a dot-product table m[n, j] = (z_t/A_t) . z0_j  (init m = x @ z0^T via
    the tensor engine; per-step rank-1 update touches only live columns j > t).
    log|det| = (D-1)*ln(P) + ln(Q) is evaluated once per class at the end, so
    the inner loop needs only the Sqrt activation table and the epilogue only
    the natural_log_exp table (one table switch total).
  * 8 cores x 13 classes (100 real + 4 padded with freq -> -1e30).
  * Epilogue: local max / sum-exp / label-masked partials, one AllToAll
    (96 KB), then each core finishes logsumexp + softmax + Dirichlet update
    for its own 1024-sample slice and writes rows [1024k, 1024(k+1)).
"""
import os
import numpy as np

import concourse.bass as bass
import concourse.bacc as bacc
import concourse.mybir as mybir
from concourse import tile
from concourse.bass_utils import run_bass_kernel_spmd
from concourse.hw_specs import get_activation_tables
from concourse.tile_rust import add_dep_helper

F = mybir.dt.float32
AF = mybir.ActivationFunctionType
OP = mybir.AluOpType
AX = mybir.AxisListType

NCORES = 8
N, D, C, T = 8192, 64, 100, 30
CP = 13            # classes per core (padded)
S = 64             # sample groups of 128 (N = 128 * S)
SL = 8             # sample groups per core in the epilogue slice
LOG_EV_CLAMP = 10.0
EV_BUDGET = 0.5 * D * float(np.log(4.0 * np.pi))
NEG_HALF_DLOG2PI = -0.5 * D * float(np.log(2.0 * np.pi))
PAD_NEGINF = -1.0e30

_CACHE = {}


def _class_split():
    """cores 0-3 get 13 real classes, cores 4-7 get 12 real + 1 pad."""
    out = []
    off = 0
    for k in range(NCORES):
        cnt = 13 if k < 4 else 12
        cls = list(range(off, off + cnt))
        off += cnt
        real = [True] * cnt
        while len(cls) < CP:
            cls.append(0)
            real.append(False)
        out.append((cls, real))
    assert off == C
    return out


def build_program():
    nc = bacc.Bacc("TRN2", target_bir_lowering=False, debug=False,
                   num_devices=NCORES)

    xaugT = nc.dram_tensor("xaugT", [D + 1, N], F, kind="ExternalInput")
    xslice = nc.dram_tensor("xslice", [D + 1, 128 * SL], F, kind="ExternalInput")
    Wb = nc.dram_tensor("Wb", [D + 1, C], F, kind="ExternalInput")
    xsq = nc.dram_tensor("xsq", [128, S], F, kind="ExternalInput")
    z0T = nc.dram_tensor("z0T", [CP, D, T], F, kind="ExternalInput")
    Gb = nc.dram_tensor("Gb", [CP, 128, T * T], F, kind="ExternalInput")
    alpha_r = nc.dram_tensor("alpha_r", [128, CP * T], F, kind="ExternalInput")
    beta_r = nc.dram_tensor("beta_r", [128, CP * T], F, kind="ExternalInput")
    n0sq_r = nc.dram_tensor("n0sq_r", [128, CP * T], F, kind="ExternalInput")
    bn_r = nc.dram_tensor("bn_r", [128, CP * T], F, kind="ExternalInput")
    ab_r = nc.dram_tensor("ab_r", [128, CP * T], F, kind="ExternalInput")
    cadd_r = nc.dram_tensor("cadd_r", [128, CP], F, kind="ExternalInput")
    corr = nc.dram_tensor("corr", [128, SL], F, kind="ExternalInput")
    masksb = nc.dram_tensor("masksb", [128, CP * S], F, kind="ExternalInput")
    out_d = nc.dram_tensor("out", [128 * SL, C + 1], F, kind="ExternalOutput")

    with tile.TileContext(nc) as tc:
        with tc.tile_pool(name="const", bufs=1) as cp_, \
             tc.tile_pool(name="gz", bufs=1) as gzp, \
             tc.tile_pool(name="mp", bufs=1) as mp_, \
             tc.tile_pool(name="st", bufs=1) as stp, \
             tc.tile_pool(name="sc", bufs=1) as sc, \
             tc.tile_pool(name="res", bufs=1) as resp, \
             tc.tile_pool(name="epi", bufs=1) as ep, \
             tc.tile_pool(name="pm", bufs=4, space="PSUM") as pmp, \
             tc.tile_pool(name="pl", bufs=4, space="PSUM") as plp, \
             tc.tile_pool(name="dram", bufs=1, space="DRAM") as dp:

            # ---- resident constants ----
            xaugT_sb = cp_.tile([D + 1, N], F)
            nc.sync.dma_start(xaugT_sb[:], xaugT[:])
            xsq_sb = cp_.tile([128, S], F)
            nc.sync.dma_start(xsq_sb[:], xsq[:])
            alpha_sb = cp_.tile([128, CP * T], F)
            nc.sync.dma_start(alpha_sb[:], alpha_r[:])
            beta_sb = cp_.tile([128, CP * T], F)
            nc.sync.dma_start(beta_sb[:], beta_r[:])
            n0sq_sb = cp_.tile([128, CP * T], F)
            nc.sync.dma_start(n0sq_sb[:], n0sq_r[:])
            bn_sb = cp_.tile([128, CP * T], F)
            nc.sync.dma_start(bn_sb[:], bn_r[:])
            ab_sb = cp_.tile([128, CP * T], F)
            nc.sync.dma_start(ab_sb[:], ab_r[:])
            cadd_sb = cp_.tile([128, CP], F)
            nc.sync.dma_start(cadd_sb[:], cadd_r[:])

            # ---- per-class results (final q, P, Q) ----
            resq = resp.tile([128, CP, S], F)
            resP = resp.tile([128, CP, S], F)
            resQ = resp.tile([128, CP, S], F)

            # =========================  flow phase  =========================
            # Classes are processed in consecutive PAIRS: all per-sample
            # scalar state lives in [128, G, S] tiles (G=2) so each update is
            # one double-width DVE op (halves per-instruction overhead), and
            # the two independent chains hide each other's serial latency.
            def group_init(cs, k, s_off=0, SS=S):
                G = len(cs)
                c0 = cs[0]
                gbp = gzp.tile([128, G * T * T], F, name=f"gbp{k}")
                nc.sync.dma_start(
                    gbp[:].rearrange("p (c g) -> p c g", c=G),
                    Gb[:].rearrange("c p g -> p c g")[:, c0:c0 + G, :])
                m = mp_.tile([128, G * SS * T], F, name=f"m{k}")
                m4 = m.rearrange("p (c s t) -> p c s t", c=G, t=T)
                for ci, c in enumerate(cs):
                    z0sb = gzp.tile([D, T], F, name=f"z0sb{k}_{ci}")
                    nc.sync.dma_start(z0sb[:], z0T[c])
                    for i in range(SS // 16):
                        pm = pmp.tile([128, 16 * T], F, name="pm")
                        for j in range(16):
                            s_ = s_off + 16 * i + j
                            nc.tensor.matmul(
                                pm[:, T * j:T * (j + 1)],
                                lhsT=xaugT_sb[0:D, 128 * s_:128 * (s_ + 1)],
                                rhs=z0sb[:], start=True, stop=True)
                        nc.scalar.copy(m4[:, ci, 16 * i:16 * (i + 1), :], pm[:])
                sl_ = slice(s_off, s_off + SS)
                q = resq[:, c0:c0 + G, sl_]
                nc.vector.tensor_copy(q, xsq_sb[:, None, sl_].broadcast_to((128, G, SS)))
                A = stp.tile([128, G, SS], F, name=f"A{k}")
                nc.gpsimd.memset(A[:], 1.0)
                nc.gpsimd.memset(resP[:, c0:c0 + G, sl_], 1.0)
                nc.gpsimd.memset(resQ[:, c0:c0 + G, sl_], 1.0)
                return dict(cs=cs, G=G, k=k, SS=SS, m4=m4,
                            gb4=gbp.rearrange("p (c t j) -> p c t j", c=G, j=T),
                            q=q, Ps=resP[:, c0:c0 + G, sl_], Qs=resQ[:, c0:c0 + G, sl_],
                            A=A)

            last_flow_act = None

            def flow_step(st, t):
                nonlocal last_flow_act
                cs, G, m4, gb4, q, Ps, Qs, A = (st["cs"], st["G"], st["m4"],
                                                st["gb4"], st["q"], st["Ps"],
                                                st["Qs"], st["A"])

                k = st["k"]
                SS = st["SS"]

                def tl(name):
                    return sc.tile([128, G, SS], F, name=f"{name}{k}")

                if t == 0:
                    dot = m4[:, :, :, 0]   # A == 1
                else:
                    dot_t = tl("dot")
                    nc.vector.tensor_tensor(out=dot_t[:], in0=A[:],
                                            in1=m4[:, :, :, t], op=OP.mult)
                    dot = dot_t[:]
                r2m = tl("r2m")
                nc.vector.scalar_tensor_tensor(r2m[:], dot, -2.0, q,
                                               op0=OP.mult, op1=OP.add)
                r = tl("r")
                s_t = tl("s_t")
                for ci, c in enumerate(cs):
                    ct = T * c + t
                    nc.scalar.activation(r[:, ci, :], r2m[:, ci, :], AF.Sqrt,
                                         bias=n0sq_sb[:, ct:ct + 1], scale=1.0)
                    nc.scalar.activation(s_t[:, ci, :], r[:, ci, :], AF.Identity,
                                         bias=alpha_sb[:, ct:ct + 1], scale=1.0)
                rs = tl("rs")
                bh = tl("bh")
                a = tl("a")
                a2 = tl("a2")
                rs2 = tl("rs2")
                v = tl("v")
                nc.vector.reciprocal_approx_fast(rs[:], s_t[:])
                nc.scalar.activation(rs2[:], rs[:], AF.Square)
                for ci, c in enumerate(cs):
                    ct = T * c + t
                    be = beta_sb[:, ct:ct + 1]
                    nc.scalar.activation(bh[:, ci, :], rs[:, ci, :], AF.Copy,
                                         scale=be)
                    nc.scalar.activation(a[:, ci, :], rs[:, ci, :], AF.Identity,
                                         bias=1.0, scale=be)
                    # a^2 = Square(beta*rs + 1); v = alpha*beta*rs^2
                    nc.scalar.activation(a2[:, ci, :], rs[:, ci, :], AF.Square,
                                         bias=1.0, scale=be)
                    last_flow_act = nc.scalar.activation(
                        v[:, ci, :], rs2[:, ci, :], AF.Copy,
                        scale=ab_sb[:, ct:ct + 1])
                # P *= a ; Q *= 1 + v
                nc.vector.tensor_tensor(out=Ps, in0=Ps, in1=a[:], op=OP.mult)
                nc.vector.scalar_tensor_tensor(Qs, v[:], 1.0, Qs,
                                               op0=OP.add, op1=OP.mult)
                # q' = a^2*q - 2*a*bh*dot + rs^2 * (beta^2*n0sq)
                u1 = tl("u1")
                nc.vector.tensor_tensor(out=u1[:], in0=bh[:], in1=dot, op=OP.mult)
                u2 = tl("u2")
                nc.vector.tensor_tensor(out=u2[:], in0=a2[:], in1=q, op=OP.mult)
                u4 = tl("u4")
                nc.vector.tensor_tensor(out=u4[:], in0=a[:], in1=u1[:], op=OP.mult)
                u3 = tl("u3")
                nc.vector.scalar_tensor_tensor(u3[:], u4[:], -2.0, u2[:],
                                               op0=OP.mult, op1=OP.add)
                for ci, c in enumerate(cs):
                    ct = T * c + t
                    nc.vector.scalar_tensor_tensor(
                        q[:, ci, :], rs2[:, ci, :], bn_sb[:, ct:ct + 1],
                        u3[:, ci, :], op0=OP.mult, op1=OP.add)
                # A *= a ; ut = bh / A'
                if t < T - 1:
                    nc.vector.tensor_tensor(out=A[:], in0=A[:], in1=a[:], op=OP.mult)
                    rA = tl("rA")
                    nc.vector.reciprocal_approx_fast(rA[:], A[:])
                    ut = tl("ut")
                    nc.vector.tensor_tensor(out=ut[:], in0=bh[:], in1=rA[:],
                                            op=OP.mult)
                    EC = 26   # E scratch cap; early steps split into 2 chunks
                    E = sc.tile([128, G * SS * EC], F, name=f"E{k}")
                    for j0 in range(t + 1, T, EC):
                        lv = min(EC, T - j0)
                        E4 = E[:, :G * SS * lv].rearrange("p (c s j) -> p c s j",
                                                          c=G, j=lv)
                        ut_b = ut[:, :, :, None].broadcast_to((128, G, SS, lv))
                        g_b = gb4[:, :, t, j0:j0 + lv][:, :, None, :].broadcast_to(
                            (128, G, SS, lv))
                        nc.vector.tensor_tensor(out=E4, in0=ut_b, in1=g_b,
                                                op=OP.mult)
                        msl = m4[:, :, :, j0:j0 + lv]
                        nc.vector.tensor_tensor(out=msl, in0=msl, in1=E4,
                                                op=OP.subtract)

            groups = [((0, 1, 2), (3, 4, 5)), ((6, 7, 8), (9, 10, 11))]
            for grp in groups:
                sts = [group_init(cs, k) for k, cs in enumerate(grp)]
                for t in range(T):
                    for st in sts:
                        flow_step(st, t)
            # odd class 12: split its samples into two parallel chains
            sts = [group_init((12,), 0, 0, S // 2),
                   group_init((12,), 1, S // 2, S // 2)]
            for t in range(T):
                for st in sts:
                    flow_step(st, t)

            # =========================  epilogue  =========================
            # Pin all epilogue ACT work after the flow phase, behind a single
            # natural_log_exp table load (Sqrt/Ln/Exp live in different sets;
            # unpinned, the scheduler interleaves them and thrashes tables).
            nle_id = list(get_activation_tables(nc.m.arch)).index(
                "natural_log_exp_and_others")
            tbl_load = mybir.InstLoadActFuncSet(
                name=f"I-{nc.next_id()}", act_func_set_id=nle_id, ins=[], outs=[])
            tl_bi = nc.scalar.add_instruction(tbl_load)
            add_dep_helper(tl_bi.ins, last_flow_act.ins, True,
                           "table load after flow phase")

            def act_pinned(out, in_, func, **kw):
                bi = nc.scalar.activation(out, in_, func, **kw)
                add_dep_helper(bi.ins, tl_bi.ins, True, "epilogue act after load")
                return bi

            # lpw_c = -0.5*q + 63*ln(P) + ln(Q) + (logfreq_c - 0.5*D*ln(2pi))
            lpw = ep.tile([128, CP * S], F)
            lpw3 = lpw.rearrange("p (c s) -> p c s", s=S)
            for c in range(CP):
                l1 = sc.tile([128, S], F)
                act_pinned(l1[:], resP[:, c, :], AF.Ln)
                l2 = sc.tile([128, S], F)
                act_pinned(l2[:], resQ[:, c, :], AF.Ln)
                w1 = sc.tile([128, S], F)
                nc.vector.scalar_tensor_tensor(w1[:], l1[:], float(D - 1), l2[:],
                                               op0=OP.mult, op1=OP.add)
                w2 = sc.tile([128, S], F)
                nc.vector.scalar_tensor_tensor(w2[:], resq[:, c, :], -0.5, w1[:],
                                               op0=OP.mult, op1=OP.add)
                nc.vector.tensor_scalar_add(lpw3[:, c, :], w2[:], cadd_sb[:, c:c + 1])

            lpw_perm = lpw.rearrange("p (c s) -> p s c", s=S)
            mx = ep.tile([128, S], F)
            nc.vector.tensor_reduce(mx[:], lpw_perm, axis=AX.X, op=OP.max)
            exs = ep.tile([128, CP * S], F)
            exs3 = exs.rearrange("p (c s) -> p c s", s=S)
            mx_b = mx[:, None, :].broadcast_to((128, CP, S))
            nc.vector.tensor_tensor(out=exs3, in0=lpw3[:, :, :], in1=mx_b, op=OP.subtract)
            act_pinned(exs[:], exs[:], AF.Exp)
            se = ep.tile([128, S], F)
            nc.vector.tensor_reduce(se[:], exs.rearrange("p (c s) -> p s c", s=S),
                                    axis=AX.X, op=OP.add)
            msk_sb = ep.tile([128, CP * S], F)
            nc.sync.dma_start(msk_sb[:], masksb[:])
            gsum = exs  # exs fully consumed by the se reduce above
            nc.vector.tensor_tensor(out=gsum[:], in0=msk_sb[:], in1=lpw[:], op=OP.mult)
            clsl = ep.tile([128, S], F)
            nc.vector.tensor_reduce(clsl[:], gsum.rearrange("p (c s) -> p s c", s=S),
                                    axis=AX.X, op=OP.add)

            # ---- AllToAll: ccin[j] = (mx, se, cls) for sample-slice j ----
            ccin = dp.tile([NCORES, 3, 128 * SL], F)
            ccout = dp.tile([NCORES, 3, 128 * SL], F)
            ccin_v = ccin.rearrange("r t (p s) -> t p r s", p=128)
            for ti, src in enumerate((mx, se, clsl)):
                nc.sync.dma_start(
                    ccin_v[ti],
                    src.rearrange("p (r s) -> p r s", s=SL))
            nc.gpsimd.collective_compute(
                "AllToAll", OP.bypass,
                replica_groups=[list(range(NCORES))],
                ins=[ccin.opt()], outs=[ccout.opt()],
            )
            ccout_v = ccout.rearrange("r t (p s) -> t p r s", p=128)
            mxg = ep.tile([128, NCORES, SL], F)
            nc.sync.dma_start(mxg[:], ccout_v[0])
            seg = ep.tile([128, NCORES, SL], F)
            nc.sync.dma_start(seg[:], ccout_v[1])
            clg = ep.tile([128, NCORES, SL], F)
            nc.sync.dma_start(clg[:], ccout_v[2])

            # ---- global combine for our slice ----
            M = ep.tile([128, SL], F)
            nc.vector.tensor_reduce(M[:], mxg.rearrange("p r s -> p s r"),
                                    axis=AX.X, op=OP.max)
            esh = ep.tile([128, NCORES * SL], F)
            esh3 = esh.rearrange("p (r s) -> p r s", s=SL)
            M_b = M[:, None, :].broadcast_to((128, NCORES, SL))
            nc.vector.tensor_tensor(out=esh3, in0=mxg[:, :, :], in1=M_b, op=OP.subtract)
            act_pinned(esh[:], esh[:], AF.Exp)
            wsum = ep.tile([128, NCORES * SL], F)
            nc.vector.tensor_tensor(out=wsum[:], in0=esh[:], in1=seg[:], op=OP.mult)
            Sg = ep.tile([128, SL], F)
            nc.vector.tensor_reduce(Sg[:], wsum.rearrange("p (r s) -> p s r", s=SL),
                                    axis=AX.X, op=OP.add)
            lse = ep.tile([128, SL], F)
            act_pinned(lse[:], Sg[:], AF.Ln)
            nc.vector.tensor_tensor(out=lse[:], in0=lse[:], in1=M[:], op=OP.add)
            clsf = ep.tile([128, SL], F)
            nc.vector.tensor_reduce(clsf[:], clg.rearrange("p r s -> p s r"),
                                    axis=AX.X, op=OP.add)
            corr_sb = ep.tile([128, SL], F)
            nc.sync.dma_start(corr_sb[:], corr[:])
            nc.vector.tensor_tensor(out=clsf[:], in0=clsf[:], in1=corr_sb[:],
                                    op=OP.subtract)
            lev = ep.tile([128, SL], F)
            nc.vector.tensor_scalar(out=lev[:], in0=lse[:], scalar1=EV_BUDGET,
                                    scalar2=LOG_EV_CLAMP, op0=OP.add, op1=OP.min)
            ev = ep.tile([128, SL], F)
            act_pinned(ev[:], lev[:], AF.Exp)

            # ---- logits + softmax + output ----
            xsl_sb = ep.tile([D + 1, 128 * SL], F)
            nc.sync.dma_start(xsl_sb[:], xslice[:])
            Wb_sb = ep.tile([D + 1, C], F)
            nc.sync.dma_start(Wb_sb[:], Wb[:])
            lg = ep.tile([128, SL * C], F)
            for j in range(SL):
                pl = plp.tile([128, C], F)
                nc.tensor.matmul(pl[:], lhsT=xsl_sb[:, 128 * j:128 * (j + 1)],
                                 rhs=Wb_sb[:], start=True, stop=True)
                nc.scalar.copy(lg[:, C * j:C * (j + 1)], pl[:])
            lg3 = lg.rearrange("p (s c) -> p s c", c=C)
            ml = ep.tile([128, SL], F)
            nc.vector.tensor_reduce(ml[:], lg3, axis=AX.X, op=OP.max)
            ml_b = ml[:, :, None].broadcast_to((128, SL, C))
            nc.vector.tensor_tensor(out=lg3, in0=lg3, in1=ml_b, op=OP.subtract)
            act_pinned(lg[:], lg[:], AF.Exp)
            ssum = ep.tile([128, SL], F)
            nc.vector.tensor_reduce(ssum[:], lg3, axis=AX.X, op=OP.add)
            rsum = ep.tile([128, SL], F)
            rscr = ep.tile([128, SL], F)
            nc.vector.reciprocal_approx_accurate(rsum[:], ssum[:], rscr[:])
            evn = ep.tile([128, SL], F)
            nc.vector.tensor_tensor(out=evn[:], in0=ev[:], in1=rsum[:], op=OP.mult)
            evn_b = evn[:, :, None].broadcast_to((128, SL, C))
            t1 = lg  # in-place: exp(logits) no longer needed afterwards
            t13 = lg3
            nc.vector.tensor_tensor(out=t13, in0=lg3, in1=evn_b, op=OP.mult)
            la = gsum[:, :SL * C]  # gsum dead after the cls reduce
            act_pinned(la[:], t1[:], AF.Ln, bias=1.0)
            # accurate log1p for small x: x*(1 + x*(-1/2 + x/3)) when x < 0.01
            h1 = ep.tile([128, SL * C], F)
            nc.vector.tensor_scalar(out=h1[:], in0=t1[:], scalar1=1.0 / 3.0,
                                    scalar2=-0.5, op0=OP.mult, op1=OP.add)
            nc.vector.tensor_tensor(out=h1[:], in0=h1[:], in1=t1[:], op=OP.mult)
            nc.vector.tensor_scalar_add(h1[:], h1[:], 1.0)
            nc.vector.tensor_tensor(out=h1[:], in0=h1[:], in1=t1[:], op=OP.mult)
            h2 = h1
            lmask = ep.tile([128, SL * C], mybir.dt.uint8)
            nc.vector.tensor_scalar(out=lmask[:], in0=t1[:], scalar1=0.01,
                                    scalar2=None, op0=OP.is_lt)
            nc.vector.select(la[:], lmask[:], h2[:], la[:])

            ob = lpw[:, :SL * (C + 1)]  # lpw dead after gsum
            ob3 = ob.rearrange("p (s c) -> p s c", c=C + 1)
            nc.vector.tensor_copy(ob3[:, :, 0:C], la.rearrange("p (s c) -> p s c", c=C))
            nc.vector.tensor_copy(ob3[:, :, C:C + 1], clsf[:, :, None])
            nc.sync.dma_start(out_d.rearrange("(s p) c -> p s c", p=128), ob3[:, :, :])

    nc.finalize()
    return nc


def _softplus(v):
    return np.log1p(np.exp(-np.abs(v))) + np.maximum(v, 0)


def host_prep(x, labels, labels_frequency, z0, alpha_prime, beta_prime, W, b):
    x = np.asarray(x, np.float32)
    labels = np.asarray(labels).astype(np.int64)
    freq = np.asarray(labels_frequency, np.float32)
    z0 = np.asarray(z0, np.float32)
    alpha = _softplus(np.asarray(alpha_prime, np.float32)).astype(np.float32)
    beta = (-alpha + _softplus(np.asarray(beta_prime, np.float32))).astype(np.float32)
    W = np.asarray(W, np.float32)
    b = np.asarray(b, np.float32)

    xaugT = np.concatenate([x.T, np.ones((1, N), np.float32)], axis=0)  # [65, N]
    Wb = np.concatenate([W, b[None, :]], axis=0).astype(np.float32)    # [65, C]
    xsq = np.sum(x * x, axis=1).astype(np.float32).reshape(S, 128).T   # [128, S]
    logfreq = np.log(freq).astype(np.float32)
    lab_ps = labels.reshape(S, 128).T                                  # [128, S]

    ones128 = np.ones((128, 1), np.float32)
    in_maps = []
    for k, (cls, real) in enumerate(_class_split()):
        z0c = z0[cls]                                   # [CP, T, D]
        alc = alpha[cls]                                # [CP, T]
        bec = beta[cls]
        G = np.einsum('cij,ckj->cik', z0c, z0c).astype(np.float32)   # [CP,T,T]
        n0 = np.sum(z0c * z0c, axis=2).astype(np.float32)            # [CP, T]
        Gb = np.broadcast_to(G.reshape(CP, 1, T * T), (CP, 128, T * T)).copy()
        alpha_rk = np.broadcast_to(alc.reshape(1, CP * T), (128, CP * T)).copy()
        beta_rk = np.broadcast_to(bec.reshape(1, CP * T), (128, CP * T)).copy()
        n0sq_rk = np.broadcast_to(n0.reshape(1, CP * T), (128, CP * T)).copy()
        bn = (bec * bec * n0).astype(np.float32)
        ab = (alc * bec).astype(np.float32)
        bn_rk = np.broadcast_to(bn.reshape(1, CP * T), (128, CP * T)).copy()
        ab_rk = np.broadcast_to(ab.reshape(1, CP * T), (128, CP * T)).copy()
        cadd = np.array([(logfreq[c] + NEG_HALF_DLOG2PI) if r else PAD_NEGINF
                         for c, r in zip(cls, real)], np.float32)
        cadd_rk = (ones128 * cadd[None, :]).astype(np.float32)
        msk = np.zeros((128, CP, S), np.float32)
        for i, (c, r) in enumerate(zip(cls, real)):
            if r:
                msk[:, i, :] = (lab_ps == c)
        sl = slice(1024 * k, 1024 * (k + 1))
        corr_k = logfreq[labels[sl]].reshape(SL, 128).T.astype(np.float32)
        in_maps.append(dict(
            xaugT=xaugT, xslice=np.ascontiguousarray(xaugT[:, sl]), Wb=Wb,
            xsq=xsq, z0T=np.ascontiguousarray(z0c.transpose(0, 2, 1)),
            Gb=Gb, alpha_r=alpha_rk, beta_r=beta_rk, n0sq_r=n0sq_rk,
            bn_r=bn_rk, ab_r=ab_rk,
            cadd_r=cadd_rk, corr=corr_k,
            masksb=msk.reshape(128, CP * S),
        ))
    return in_maps


def kernel(**inputs) -> np.ndarray:
    if "nc" not in _CACHE:
        _CACHE["nc"] = build_program()
    nc = _CACHE["nc"]
    in_maps = host_prep(**inputs)
    if os.environ.get("KERNEL_SIM"):
        from concourse.bass_interp import MultiCoreSim
        sim = MultiCoreSim(nc, NCORES)
        for k in range(NCORES):
            for name, arr in in_maps[k].items():
                sim.cores[k].tensor(name)[:] = arr
        sim.simulate()
        outs = [np.array(sim.cores[k].tensor("out")) for k in range(NCORES)]
    else:
        res = run_bass_kernel_spmd(nc, in_maps, list(range(NCORES)))
        outs = [res.results[k]["out"] for k in range(NCORES)]
    return np.concatenate(outs, axis=0)



# revision 62
# speedup vs baseline: 1.8995x; 1.8995x over previous
"""NatPN radial-flow posterior kernel for Trainium2, 8 NeuronCores (SPMD).

Strategy (class-sharded "expert parallel", lazy-correction flow):
  * Radial flow z' = z + beta*h*(z - z0), h = 1/(alpha + r), r = |z - z0| is
    run in coefficient space z_t = A_t * w_t, w_t = x - sum_k u_k z0_k.
    Per step we need only the scalar dot d1 = w . z0_t, recovered LAZILY:
        d1_t = m0[t] - sum_{k<t} u_k G[k, t]
    with m0 = x @ z0^T (tensor engine) and G = z0 @ z0^T (host). The inner
    product runs as one bf16 2x-mode multiply (u-history and G rows both have
    packed last dims) plus a bf16 add-tree + small tensor_reduce — about half
    the vector-engine cost of eagerly maintaining the m table.
  * State per (class, sample): qw = |w|^2, A = prod(1+bh) (which IS the P
    determinant product), Q = prod(1+alpha*beta*h^2). The qw/Q updates run on
    the otherwise-idle GPSIMD engine; r = sqrt(...) on the scalar engine.
  * 8 cores x 13 classes (100 real + 4 padded with freq -> -1e30), classes
    split into two pipelined groups (7+6) so engines overlap.
  * Epilogue: local max / sum-exp / label-masked partials, one AllToAll
    (96 KB), then each core finishes logsumexp + softmax + Dirichlet update
    for its own 1024-sample slice and writes rows [1024k, 1024(k+1)).
"""
import os
import numpy as np

import concourse.bass as bass
import concourse.bacc as bacc
import concourse.mybir as mybir
from concourse import tile
from concourse.bass_utils import run_bass_kernel_spmd
from concourse.hw_specs import get_activation_tables
from concourse.tile_rust import add_dep_helper

F = mybir.dt.float32
BF = mybir.dt.bfloat16
AF = mybir.ActivationFunctionType
OP = mybir.AluOpType
AX = mybir.AxisListType

NCORES = 8
N, D, C, T = 8192, 64, 100, 30
CP = 13            # classes per core (padded)
S = 64             # sample groups of 128 (N = 128 * S)
SL = 8             # sample groups per core in the epilogue slice
LOG_EV_CLAMP = 10.0
EV_BUDGET = 0.5 * D * float(np.log(4.0 * np.pi))
NEG_HALF_DLOG2PI = -0.5 * D * float(np.log(2.0 * np.pi))
TE = 6             # steps below this keep chain ops on DVE (latency-bound)
PAD_NEGINF = -1.0e30

_CACHE = {}


def _class_split():
    """cores 0-3 get 13 real classes, cores 4-7 get 12 real + 1 pad."""
    out = []
    off = 0
    for k in range(NCORES):
        cnt = 13 if k < 4 else 12
        cls = list(range(off, off + cnt))
        off += cnt
        real = [True] * cnt
        while len(cls) < CP:
            cls.append(0)
            real.append(False)
        out.append((cls, real))
    assert off == C
    return out


def build_program():
    nc = bacc.Bacc("TRN2", target_bir_lowering=False, debug=False,
                   num_devices=NCORES)

    xbf = nc.dram_tensor("xbf", [D, N], BF, kind="ExternalInput")
    xslice = nc.dram_tensor("xslice", [D + 1, 128 * SL], F, kind="ExternalInput")
    Wb = nc.dram_tensor("Wb", [D + 1, C], F, kind="ExternalInput")
    xsq = nc.dram_tensor("xsq", [128, S], F, kind="ExternalInput")
    z0T = nc.dram_tensor("z0T", [D, CP * T], BF, kind="ExternalInput")
    Gb = nc.dram_tensor("Gb", [CP, 128, T * T], BF, kind="ExternalInput")
    # alpha | beta | n0sq | ab stacked into one tensor -> one DMA
    tabs_r = nc.dram_tensor("tabs_r", [128, 4 * CP * T], F, kind="ExternalInput")
    cadd_r = nc.dram_tensor("cadd_r", [128, CP], F, kind="ExternalInput")
    corr_in = nc.dram_tensor("corr", [128, SL], F, kind="ExternalInput")
    masksb = nc.dram_tensor("masksb", [128, CP * S], F, kind="ExternalInput")
    out_d = nc.dram_tensor("out", [128 * SL, C + 1], F, kind="ExternalOutput")

    GROUPS = [(0, 7), (7, 6)]   # (class offset, count)

    with tile.TileContext(nc) as tc:
        with tc.tile_pool(name="const", bufs=1) as cp_, \
             tc.tile_pool(name="st", bufs=1) as stp, \
             tc.tile_pool(name="pm", bufs=6, space="PSUM") as pmp, \
             tc.tile_pool(name="pl", bufs=2, space="PSUM") as plp, \
             tc.tile_pool(name="dram", bufs=1, space="DRAM") as dp:

            # ---- resident constants / state ----
            # (DMA issue order matters: x chunk + z0 first so the tensor
            #  engine starts immediately; big Gb load afterwards)
            tabs_sb = cp_.tile([128, 4 * CP * T], F)
            CT = CP * T
            alpha_sb = tabs_sb[:, 0:CT]
            beta_sb = tabs_sb[:, CT:2 * CT]
            n0sq_sb = tabs_sb[:, 2 * CT:3 * CT]
            ab_sb = tabs_sb[:, 3 * CT:4 * CT]
            xsq_sb = cp_.tile([128, S], F)
            cadd_sb = cp_.tile([128, CP], F)
            gb_sb = cp_.tile([128, CP * T * T], BF)
            gb4 = gb_sb.rearrange("p (c t j) -> p c t j", c=CP, j=T)

            # per-(c,s) f32 state, CP-wide (groups use class sub-ranges)
            qw_t = stp.tile([128, CP, S], F)
            A_t = stp.tile([128, CP, S], F)
            Q_t = stp.tile([128, CP, S], F)
            d1_t = stp.tile([128, CP, S], F)
            rs_t = stp.tile([128, CP, S], F)
            bh_t = stp.tile([128, CP, S], F)
            rs2_t = stp.tile([128, CP, S], F)

            last_flow_act = None

            with tc.tile_pool(name="big", bufs=1) as bigp, \
                 tc.tile_pool(name="sc", bufs=1) as sc:
                m0 = bigp.tile([128, T * CP * S], BF)        # t-major: [t, c, s]
                m04 = m0.rearrange("p (t c s) -> p t c s", t=T, s=S)
                uacc = bigp.tile([128, CP * S * T], BF)      # k-minor: [c, s, k]
                uacc4 = uacc.rearrange("p (c s k) -> p c s k", s=S, k=T)
                wtl = bigp.tile([128, 7 * S * (T - 1)], BF)  # shared scratch

                # ---- init: m0 = x @ z0^T on the tensor engine ----
                # x loaded in 2048-sample chunks; group A's classes complete
                # first so its flow steps start while group B still inits.
                with tc.tile_pool(name="init", bufs=1) as ip:
                    xch0 = ip.tile([D, 128 * 16], BF, name="xch0")
                    nc.sync.dma_start(xch0[:], xbf[:, 0:2048])
                    z0all = ip.tile([D, CP * T], BF)
                    nc.sync.dma_start(z0all[:], z0T[:])
                    # remaining constants (needed only once flow starts)
                    nc.sync.dma_start(tabs_sb[:], tabs_r[:])
                    nc.sync.dma_start(xsq_sb[:], xsq[:])
                    nc.sync.dma_start(cadd_sb[:], cadd_r[:])
                    nc.sync.dma_start(
                        gb_sb[:].rearrange("p (c g) -> p c g", c=CP),
                        Gb[:].rearrange("c p g -> p c g"))
                    cp_rr = [0]
                    for (c0g, Gg) in GROUPS:
                        # class blocks of up to 4 share one rhs (wider
                        # matmuls -> 3.5x fewer PE instructions)
                        blocks = []
                        cc = c0g
                        while cc < c0g + Gg:
                            nb = min(4, c0g + Gg - cc)
                            blocks.append((cc, nb))
                            cc += nb
                        for i in range(S // 16):
                            if (c0g, i) == (0, 0):
                                xch = xch0
                            else:
                                xch = ip.tile([D, 128 * 16], BF,
                                              name=f"xch{i % 2}")
                                nc.sync.dma_start(
                                    xch[:], xbf[:, 2048 * i:2048 * (i + 1)])
                            for (cb, nb) in blocks:
                                w_ = T * nb          # rhs cols
                                jm = 480 // w_       # samples per PSUM tile
                                for j0 in range(0, 16, jm):
                                    jn = min(jm, 16 - j0)
                                    pm = pmp.tile([128, jm * w_], F, name="pm")
                                    for j in range(jn):
                                        s_ = 128 * (j0 + j)
                                        nc.tensor.matmul(
                                            pm[:, w_ * j:w_ * (j + 1)],
                                            lhsT=xch[:, s_:s_ + 128],
                                            rhs=z0all[:, T * cb:T * cb + w_],
                                            start=True, stop=True)
                                    # pm dims (j, c, t) -> m0[t, c, 16i+j0+j]
                                    # copies round-robin across DVE/ACT/Pool
                                    # (all three idle during init)
                                    s0 = 16 * i + j0
                                    cp_fn = (nc.vector.tensor_copy,
                                             nc.scalar.copy)[cp_rr[0] % 2]
                                    cp_rr[0] += 1
                                    cp_fn(
                                        m04[:, :, cb:cb + nb,
                                            s0:s0 + jn].rearrange(
                                                "p t c s -> p s c t"),
                                        pm[:, :jn * w_].rearrange(
                                            "p (s c t) -> p s c t",
                                            c=nb, t=T))

                # state init
                nc.vector.tensor_copy(
                    qw_t[:, :, :],
                    xsq_sb[:, None, :].broadcast_to((128, CP, S)))
                nc.gpsimd.memset(A_t[:], 1.0)
                nc.gpsimd.memset(Q_t[:], 1.0)

                # =======================  flow phase  =======================
                # Lookahead correction: during step t we precompute
                #   corrpre(t+1) = sum_{k<t} u_k * G[k, t+1]
                # (bulk multiply + bf16 add-tree, off the serial chain) and
                #   s1(t+1) = m0[t+1] - corrpre(t+1).
                # The inter-step chain then only carries the last column:
                #   d1(t+1) = s1(t+1) - u_t * G[t, t+1].
                # The r^2 / qw / Q scalar chains run on the GPSIMD engine.
                gstate = [dict() for _ in GROUPS]

                def gslice(g):
                    c0, G_ = GROUPS[g]
                    return (slice(None), slice(c0, c0 + G_), slice(None))

                def flow_head(g, t):
                    c0, G_ = GROUPS[g]
                    st = gstate[g]
                    FGS = G_ * S
                    d1 = d1_t[gslice(g)]
                    m0col = m04[:, t, c0:c0 + G_, :]
                    u4 = uacc4[:, c0:c0 + G_, :, :]

                    def tl(name):
                        return sc.tile([128, G_, S], F, name=f"{name}{g}")

                    # ---- corr = sum_{k<t} u_k * G[k, t]  (lazy dot) ----
                    if t == 0:
                        nc.vector.tensor_copy(d1, m0col)
                    else:
                        w4 = wtl[:, :FGS * t].rearrange(
                            "p (c s k) -> p c s k", c=G_, k=t)
                        gview = gb4[:, c0:c0 + G_, t, 0:t][:, :, None, :]
                        nc.vector.tensor_tensor(
                            out=w4, in0=u4[:, :, :, 0:t],
                            in1=gview.broadcast_to((128, G_, S, t)), op=OP.mult)
                        k = t
                        if k > 2:
                            p2 = 1 << (k.bit_length() - 1)
                            if p2 == k:
                                p2 //= 2
                            nc.vector.tensor_tensor(
                                out=w4[:, :, :, 0:k - p2],
                                in0=w4[:, :, :, 0:k - p2],
                                in1=w4[:, :, :, p2:k], op=OP.add)
                            k = p2
                            while k > 2:
                                h = k // 2
                                nc.vector.tensor_tensor(
                                    out=w4[:, :, :, 0:h], in0=w4[:, :, :, 0:h],
                                    in1=w4[:, :, :, h:k], op=OP.add)
                                k = h
                        nc.vector.tensor_tensor(
                            out=d1, in0=m0col, in1=w4[:, :, :, 0],
                            op=OP.subtract)
                        if k == 2:
                            nc.vector.tensor_tensor(
                                out=d1, in0=d1, in1=w4[:, :, :, 1],
                                op=OP.subtract)
                    dd = tl("dd")
                    nc.gpsimd.tensor_tensor(out=dd[:], in0=d1, in1=d1,
                                            op=OP.add)
                    st["dd"] = dd

                def flow_mid(g, t):
                    nonlocal last_flow_act
                    c0, G_ = GROUPS[g]
                    st = gstate[g]
                    dd = st["dd"]
                    sl3 = gslice(g)
                    qw = qw_t[sl3]
                    A = A_t[sl3]
                    Q = Q_t[sl3]
                    d1 = d1_t[sl3]
                    rs = rs_t[sl3]
                    bh = bh_t[sl3]
                    rs2 = rs2_t[sl3]
                    u4 = uacc4[:, c0:c0 + G_, :, :]

                    def tl(name):
                        return sc.tile([128, G_, S], F, name=f"{name}{g}")

                    # ---- r2 = A*(A*qw - 2*d1) + n0sq ----
                    g1 = tl("g1")
                    nc.vector.tensor_tensor(out=g1[:], in0=A, in1=qw,
                                            op=OP.mult)
                    g2 = tl("g2")
                    nc.vector.scalar_tensor_tensor(g2[:], d1, -2.0, g1[:],
                                                   op0=OP.mult, op1=OP.add)
                    r2m = tl("g1")      # g1 dead after g2
                    nc.vector.tensor_tensor(out=r2m[:], in0=A, in1=g2[:],
                                            op=OP.mult)
                    r = tl("lc")        # lc dead after d1
                    for ci in range(G_):
                        ct = T * (c0 + ci) + t
                        last_flow_act = nc.scalar.activation(
                            r[:, ci, :], r2m[:, ci, :], AF.Sqrt,
                            bias=n0sq_sb[:, ct:ct + 1], scale=1.0)
                    # s = r + alpha ; rs = 1/s ; bh = beta*rs
                    # early steps are chain-bound: keep s_t on DVE there
                    s_t = tl("s_t")
                    if t < TE:
                        av = alpha_sb.rearrange("p (c t) -> p c t", t=T)[
                            :, c0:c0 + G_, t][:, :, None]
                        nc.vector.tensor_tensor(
                            out=s_t[:], in0=r[:],
                            in1=av.broadcast_to((128, G_, S)), op=OP.add)
                    else:
                        for ci in range(G_):
                            ct = T * (c0 + ci) + t
                            nc.scalar.activation(
                                s_t[:, ci, :], r[:, ci, :], AF.Identity,
                                bias=alpha_sb[:, ct:ct + 1], scale=1.0)
                    nc.vector.reciprocal_approx_fast(rs, s_t[:])
                    bv = beta_sb.rearrange("p (c t) -> p c t", t=T)[
                        :, c0:c0 + G_, t][:, :, None]
                    nc.vector.tensor_tensor(
                        out=bh, in0=rs,
                        in1=bv.broadcast_to((128, G_, S)), op=OP.mult)
                    # Q *= 1 + ab*rs^2   (scalar engine square, gpsimd chain)
                    nc.scalar.activation(rs2, rs, AF.Square)
                    k1 = tl("s_t")      # s_t dead after rs
                    abv = ab_sb.rearrange("p (c t) -> p c t", t=T)[
                        :, c0:c0 + G_, t][:, :, None]
                    nc.gpsimd.tensor_tensor(
                        out=k1[:], in0=rs2,
                        in1=abv.broadcast_to((128, G_, S)), op=OP.mult)
                    # Q *= (1 + v)  as  Q += Q*v  (no scalar ops on Pool)
                    k2 = tl("g1")
                    nc.gpsimd.tensor_tensor(out=k2[:], in0=Q, in1=k1[:],
                                            op=OP.mult)
                    nc.gpsimd.tensor_tensor(out=Q, in0=Q, in1=k2[:], op=OP.add)
                    # A' = (1+bh)*A  (in place)
                    nc.vector.scalar_tensor_tensor(A, bh, 1.0, A,
                                                   op0=OP.add, op1=OP.mult)
                    # ut = bh / A'  (stored bf16 into the u history)
                    rA = tl("g2")       # g2 dead after r2m
                    nc.vector.reciprocal_approx_fast(rA[:], A)
                    ut = u4[:, :, :, t]
                    nc.vector.tensor_tensor(out=ut, in0=bh, in1=rA[:],
                                            op=OP.mult)
                    # qw' = qw + ut*(ut*Gtt - 2*d1)
                    # (gpsimd once steps are long enough to hide it)
                    qe = nc.gpsimd
                    gttv = gb4[:, c0:c0 + G_, t, t][:, :, None]
                    h1 = tl("h1")
                    qe.tensor_tensor(
                        out=h1[:], in0=ut,
                        in1=gttv.broadcast_to((128, G_, S)), op=OP.mult)
                    h2 = tl("h2")
                    qe.tensor_tensor(out=h2[:], in0=h1[:], in1=dd[:],
                                     op=OP.subtract)
                    h3 = tl("h1")       # h1 dead after h2
                    qe.tensor_tensor(out=h3[:], in0=ut, in1=h2[:],
                                     op=OP.mult)
                    qe.tensor_tensor(out=qw, in0=qw, in1=h3[:],
                                     op=OP.add)

                for t in range(T):
                    for g in range(len(GROUPS)):
                        flow_head(g, t)
                    for g in range(len(GROUPS)):
                        flow_mid(g, t)

            # =========================  epilogue  =========================
            # Pin all epilogue ACT work behind a single natural_log_exp table
            # load (Sqrt/Ln/Exp live in different sets).
            nle_id = list(get_activation_tables(nc.m.arch)).index(
                "natural_log_exp_and_others")
            tbl_load = mybir.InstLoadActFuncSet(
                name=f"I-{nc.next_id()}", act_func_set_id=nle_id, ins=[], outs=[])
            tl_bi = nc.scalar.add_instruction(tbl_load)
            add_dep_helper(tl_bi.ins, last_flow_act.ins, True,
                           "table load after flow phase")

            def act_pinned(out, in_, func, **kw):
                bi = nc.scalar.activation(out, in_, func, **kw)
                add_dep_helper(bi.ins, tl_bi.ins, True, "epilogue act after load")
                return bi

            with tc.tile_pool(name="epi", bufs=1) as ep:
                lpw = ep.tile([128, CP * S], F)
                lpw3 = lpw.rearrange("p (c s) -> p c s", s=S)
                # lpw = -0.5*A^2*qw + 63*ln(A) + ln(Q) + cadd
                # za/zq/zqc only need flow state -> gpsimd, ahead of the Lns
                za = ep.tile([128, CP, S], F)
                nc.vector.tensor_tensor(out=za[:, :, :], in0=A_t[:, :, :],
                                        in1=A_t[:, :, :], op=OP.mult)
                zq = ep.tile([128, CP, S], F)
                nc.vector.tensor_tensor(out=zq[:, :, :], in0=za[:, :, :],
                                        in1=qw_t[:, :, :], op=OP.mult)
                cv = cadd_sb[:, :, None]
                zqc = ep.tile([128, CP, S], F)
                nc.vector.scalar_tensor_tensor(
                    zqc[:, :, :], zq[:, :, :], -0.5,
                    cv.broadcast_to((128, CP, S)),
                    op0=OP.mult, op1=OP.add)
                l1 = ep.tile([128, CP * S], F)
                act_pinned(l1[:], A_t[:, :, :].rearrange("p c s -> p (c s)"),
                           AF.Ln)
                l2 = ep.tile([128, CP * S], F)
                act_pinned(l2[:], Q_t[:, :, :].rearrange("p c s -> p (c s)"),
                           AF.Ln)
                w1 = ep.tile([128, CP * S], F)
                nc.vector.scalar_tensor_tensor(w1[:], l1[:], float(D - 1), l2[:],
                                               op0=OP.mult, op1=OP.add)
                nc.vector.tensor_tensor(
                    out=lpw3, in0=w1.rearrange("p (c s) -> p c s", s=S),
                    in1=zqc[:, :, :], op=OP.add)

                lpw_perm = lpw.rearrange("p (c s) -> p s c", s=S)
                mx = ep.tile([128, S], F)
                nc.vector.tensor_reduce(mx[:], lpw_perm, axis=AX.X, op=OP.max)
                exs = ep.tile([128, CP * S], F)
                exs3 = exs.rearrange("p (c s) -> p c s", s=S)
                mx_b = mx[:, None, :].broadcast_to((128, CP, S))
                nc.vector.tensor_tensor(out=exs3, in0=lpw3[:, :, :], in1=mx_b,
                                        op=OP.subtract)
                act_pinned(exs[:], exs[:], AF.Exp)
                se = ep.tile([128, S], F)
                nc.vector.tensor_reduce(
                    se[:], exs.rearrange("p (c s) -> p s c", s=S),
                    axis=AX.X, op=OP.add)
                msk_sb = ep.tile([128, CP * S], F)
                nc.sync.dma_start(msk_sb[:], masksb[:])
                gsum = exs  # exs fully consumed by the se reduce above
                nc.vector.tensor_tensor(out=gsum[:], in0=msk_sb[:], in1=lpw[:],
                                        op=OP.mult)
                clsl = ep.tile([128, S], F)
                nc.vector.tensor_reduce(
                    clsl[:], gsum.rearrange("p (c s) -> p s c", s=S),
                    axis=AX.X, op=OP.add)

                # ---- AllToAll: ccin[j] = (mx, se, cls) for sample-slice j ----
                ccin = dp.tile([NCORES, 3, 128 * SL], F)
                ccout = dp.tile([NCORES, 3, 128 * SL], F)
                ccin_v = ccin.rearrange("r t (p s) -> t p r s", p=128)
                for ti, src in enumerate((mx, se, clsl)):
                    nc.sync.dma_start(
                        ccin_v[ti],
                        src.rearrange("p (r s) -> p r s", s=SL))
                nc.gpsimd.collective_compute(
                    "AllToAll", OP.bypass,
                    replica_groups=[list(range(NCORES))],
                    ins=[ccin.opt()], outs=[ccout.opt()],
                )
                # ---- logits path fills the AllToAll wait ----
                xsl_sb = ep.tile([D + 1, 128 * SL], F)
                nc.sync.dma_start(xsl_sb[:], xslice[:])
                Wb_sb = ep.tile([D + 1, C], F)
                nc.sync.dma_start(Wb_sb[:], Wb[:])
                lg = ep.tile([128, SL * C], F)
                for j in range(SL):
                    pl = plp.tile([128, C], F)
                    nc.tensor.matmul(pl[:],
                                     lhsT=xsl_sb[:, 128 * j:128 * (j + 1)],
                                     rhs=Wb_sb[:], start=True, stop=True)
                    nc.scalar.copy(lg[:, C * j:C * (j + 1)], pl[:])
                lg3 = lg.rearrange("p (s c) -> p s c", c=C)
                ml = ep.tile([128, SL], F)
                nc.vector.tensor_reduce(ml[:], lg3, axis=AX.X, op=OP.max)
                ml_b = ml[:, :, None].broadcast_to((128, SL, C))
                nc.vector.tensor_tensor(out=lg3, in0=lg3, in1=ml_b,
                                        op=OP.subtract)
                act_pinned(lg[:], lg[:], AF.Exp)
                ssum = ep.tile([128, SL], F)
                nc.vector.tensor_reduce(ssum[:], lg3, axis=AX.X, op=OP.add)
                rsum = ep.tile([128, SL], F)
                rscr = ep.tile([128, SL], F)
                nc.vector.reciprocal_approx_accurate(rsum[:], ssum[:], rscr[:])

                ccout_v = ccout.rearrange("r t (p s) -> t p r s", p=128)
                mxg = ep.tile([128, NCORES, SL], F)
                nc.sync.dma_start(mxg[:], ccout_v[0])
                seg = ep.tile([128, NCORES, SL], F)
                nc.sync.dma_start(seg[:], ccout_v[1])
                clg = ep.tile([128, NCORES, SL], F)
                nc.sync.dma_start(clg[:], ccout_v[2])

                # ---- global combine for our slice ----
                M = ep.tile([128, SL], F)
                nc.vector.tensor_reduce(M[:], mxg.rearrange("p r s -> p s r"),
                                        axis=AX.X, op=OP.max)
                esh = ep.tile([128, NCORES * SL], F)
                esh3 = esh.rearrange("p (r s) -> p r s", s=SL)
                M_b = M[:, None, :].broadcast_to((128, NCORES, SL))
                nc.vector.tensor_tensor(out=esh3, in0=mxg[:, :, :], in1=M_b,
                                        op=OP.subtract)
                act_pinned(esh[:], esh[:], AF.Exp)
                wsum = ep.tile([128, NCORES * SL], F)
                nc.vector.tensor_tensor(out=wsum[:], in0=esh[:], in1=seg[:],
                                        op=OP.mult)
                Sg = ep.tile([128, SL], F)
                nc.vector.tensor_reduce(
                    Sg[:], wsum.rearrange("p (r s) -> p s r", s=SL),
                    axis=AX.X, op=OP.add)
                lse = ep.tile([128, SL], F)
                act_pinned(lse[:], Sg[:], AF.Ln)
                nc.vector.tensor_tensor(out=lse[:], in0=lse[:], in1=M[:],
                                        op=OP.add)
                clsf = ep.tile([128, SL], F)
                nc.vector.tensor_reduce(clsf[:], clg.rearrange("p r s -> p s r"),
                                        axis=AX.X, op=OP.add)
                corr_sb = ep.tile([128, SL], F)
                nc.sync.dma_start(corr_sb[:], corr_in[:])
                nc.vector.tensor_tensor(out=clsf[:], in0=clsf[:], in1=corr_sb[:],
                                        op=OP.subtract)
                lev = ep.tile([128, SL], F)
                nc.vector.tensor_scalar(out=lev[:], in0=lse[:],
                                        scalar1=EV_BUDGET,
                                        scalar2=LOG_EV_CLAMP, op0=OP.add,
                                        op1=OP.min)
                ev = ep.tile([128, SL], F)
                act_pinned(ev[:], lev[:], AF.Exp)

                # ---- combine evidence with precomputed softmax ----
                evn = ep.tile([128, SL], F)
                nc.vector.tensor_tensor(out=evn[:], in0=ev[:], in1=rsum[:],
                                        op=OP.mult)
                evn_b = evn[:, :, None].broadcast_to((128, SL, C))
                t1 = lg  # in-place: exp(logits) no longer needed afterwards
                t13 = lg3
                nc.vector.tensor_tensor(out=t13, in0=lg3, in1=evn_b, op=OP.mult)
                la = gsum[:, :SL * C]  # gsum dead after the cls reduce
                act_pinned(la[:], t1[:], AF.Ln, bias=1.0)
                # accurate log1p for small x: x*(1 + x*(-1/2 + x/3)) when x<0.01
                h1e = ep.tile([128, SL * C], F)
                nc.vector.tensor_scalar(out=h1e[:], in0=t1[:], scalar1=1.0 / 3.0,
                                        scalar2=-0.5, op0=OP.mult, op1=OP.add)
                nc.vector.tensor_tensor(out=h1e[:], in0=h1e[:], in1=t1[:],
                                        op=OP.mult)
                nc.vector.tensor_scalar_add(h1e[:], h1e[:], 1.0)
                nc.vector.tensor_tensor(out=h1e[:], in0=h1e[:], in1=t1[:],
                                        op=OP.mult)
                h2e = h1e
                lmask = ep.tile([128, SL * C], mybir.dt.uint8)
                nc.vector.tensor_scalar(out=lmask[:], in0=t1[:], scalar1=0.01,
                                        scalar2=None, op0=OP.is_lt)
                nc.vector.select(la[:], lmask[:], h2e[:], la[:])

                ob = lpw[:, :SL * (C + 1)]  # lpw dead after gsum
                ob3 = ob.rearrange("p (s c) -> p s c", c=C + 1)
                nc.vector.tensor_copy(ob3[:, :, 0:C],
                                      la.rearrange("p (s c) -> p s c", c=C))
                nc.vector.tensor_copy(ob3[:, :, C:C + 1], clsf[:, :, None])
                nc.sync.dma_start(out_d.rearrange("(s p) c -> p s c", p=128),
                                  ob3[:, :, :])

    nc.finalize()
    return nc


def _softplus(v):
    return np.log1p(np.exp(-np.abs(v))) + np.maximum(v, 0)


def host_prep(x, labels, labels_frequency, z0, alpha_prime, beta_prime, W, b):
    import ml_dtypes
    x = np.asarray(x, np.float32)
    labels = np.asarray(labels).astype(np.int64)
    freq = np.asarray(labels_frequency, np.float32)
    z0 = np.asarray(z0, np.float32)
    alpha = _softplus(np.asarray(alpha_prime, np.float32)).astype(np.float32)
    beta = (-alpha + _softplus(np.asarray(beta_prime, np.float32))).astype(np.float32)
    W = np.asarray(W, np.float32)
    b = np.asarray(b, np.float32)

    xaugT = np.concatenate([x.T, np.ones((1, N), np.float32)], axis=0)  # [65, N]
    xbf = np.ascontiguousarray(x.T).astype(ml_dtypes.bfloat16)          # [D, N]
    Wb = np.concatenate([W, b[None, :]], axis=0).astype(np.float32)    # [65, C]
    xsq = np.sum(x * x, axis=1).astype(np.float32).reshape(S, 128).T   # [128, S]
    logfreq = np.log(freq).astype(np.float32)
    lab_ps = labels.reshape(S, 128).T                                  # [128, S]

    ones128 = np.ones((128, 1), np.float32)
    in_maps = []
    for k, (cls, real) in enumerate(_class_split()):
        z0c = z0[cls]                                   # [CP, T, D]
        alc = alpha[cls]                                # [CP, T]
        bec = beta[cls]
        G = np.einsum('cij,ckj->cik', z0c, z0c).astype(np.float32)   # [CP,T,T]
        n0 = np.sum(z0c * z0c, axis=2).astype(np.float32)            # [CP, T]
        Gb = np.broadcast_to(
            G.astype(ml_dtypes.bfloat16).reshape(CP, 1, T * T),
            (CP, 128, T * T)).copy()
        ab = (alc * bec).astype(np.float32)
        tabs = np.concatenate([alc.reshape(-1), bec.reshape(-1),
                               n0.reshape(-1), ab.reshape(-1)])
        tabs_rk = np.broadcast_to(tabs.reshape(1, 4 * CP * T),
                                  (128, 4 * CP * T)).copy()
        cadd = np.array([(logfreq[c] + NEG_HALF_DLOG2PI) if r else PAD_NEGINF
                         for c, r in zip(cls, real)], np.float32)
        cadd_rk = (ones128 * cadd[None, :]).astype(np.float32)
        msk = np.zeros((128, CP, S), np.float32)
        for i, (c, r) in enumerate(zip(cls, real)):
            if r:
                msk[:, i, :] = (lab_ps == c)
        sl = slice(1024 * k, 1024 * (k + 1))
        corr_k = logfreq[labels[sl]].reshape(SL, 128).T.astype(np.float32)
        in_maps.append(dict(
            xbf=xbf, xslice=np.ascontiguousarray(xaugT[:, sl]), Wb=Wb,
            xsq=xsq,
            z0T=np.ascontiguousarray(z0c.transpose(2, 0, 1)).reshape(
                D, CP * T).astype(ml_dtypes.bfloat16),
            Gb=Gb, tabs_r=tabs_rk,
            cadd_r=cadd_rk, corr=corr_k,
            masksb=msk.reshape(128, CP * S),
        ))
    return in_maps


def kernel(**inputs) -> np.ndarray:
    if "nc" not in _CACHE:
        _CACHE["nc"] = build_program()
    nc = _CACHE["nc"]
    in_maps = host_prep(**inputs)
    if os.environ.get("KERNEL_SIM"):
        from concourse.bass_interp import MultiCoreSim
        sim = MultiCoreSim(nc, NCORES)
        for k in range(NCORES):
            for name, arr in in_maps[k].items():
                sim.cores[k].tensor(name)[:] = arr
        sim.simulate()
        outs = [np.array(sim.cores[k].tensor("out")) for k in range(NCORES)]
    else:
        res = run_bass_kernel_spmd(nc, in_maps, list(range(NCORES)))
        outs = [res.results[k]["out"] for k in range(NCORES)]
    return np.concatenate(outs, axis=0)
